# revision 7
# baseline (speedup 1.0000x reference)
"""Two-network GCN encoder + edge decoder on 8 TRN2 NeuronCores (Bass/Tile).

Math: GCNConv with symmetric norm factorizes as
    out = dis * (scatter_add(h'[src] by dst) + h') + b,   h' = dis * (x @ W)
with dis = (deg+1)^-1/2 per node, so all per-edge work is unweighted
gather + add; dis is a diagonal scaling applied in matmul epilogues.

Sharding: nodes row-sharded 12500/core (padded to 12544 rows/shard; pad rows
are zeros and double as zero-source rows for gather padding).  Edges live on
the core owning their dst.  Per conv layer: sharded dense matmul ->
AllGather of the per-node message table -> per-src-bucket dma_gather
(int16 indices; 4 buckets of 25088 rows) into a degree-padded grid ->
on-chip segment reduction (tensor_reduce) -> dma_scatter_add of per-bucket
unique dst rows (unique within a call => no RMW races; pad entries add
zeros to scratch rows).  The decoder reuses the same grids and index
arrays against the AllGather'd z table; dst-side rows come from the local
shard bounce.
"""
import sys
sys.path.insert(0, '/opt/trn_rl_repo')
import numpy as np

N = 100000
E = 500000
NSH = 12500          # real nodes per shard
SHP = 12544          # padded shard rows (98 * 128)
NCORES = 8
NTOT = SHP * NCORES  # 100352
BUCK = 2 * SHP       # 25088 rows per gather bucket (int16-safe)
NBUCK = 4
ZREL = 12500         # a zero row relative to any bucket/shard base
GROUPS = SHP // 128  # 98 node tiles per shard
MAXCOL_CALL = 24     # gather cols per call (<= 3072 idxs)
PADP = NSH - (NSH // 128) * 128   # 84: first pad partition in last tile

_cache = {}
TRACE = False
LAST_EXEC_NS = [None]


# ----------------------------------------------------------------- host prep
def _prep_graph(edge_index):
    src = np.asarray(edge_index[0], dtype=np.int64)
    dst = np.asarray(edge_index[1], dtype=np.int64)
    deg = np.bincount(dst, minlength=N).astype(np.float32) + 1.0
    core = dst // NSH
    d_l = dst - core * NSH
    srow = (src // NSH) * SHP + (src % NSH)
    sb = srow // BUCK
    srel = (srow - sb * BUCK).astype(np.int64)
    per_core = []
    for c in range(NCORES):
        m = core == c
        per_core.append(_prep_core(d_l[m], sb[m], srel[m], np.nonzero(m)[0]))
    return deg, per_core


def _prep_core(d_l, sb, srel, eid):
    buckets = []
    for b in range(NBUCK):
        m = sb == b
        bd, bs, be = d_l[m], srel[m], eid[m]
        order = np.lexsort((be, bd))
        bd, bs, be = bd[order], bs[order], be[order]
        uniq, start, cnt = np.unique(bd, return_index=True, return_counts=True)
        o2 = np.argsort(-cnt, kind="stable")
        buckets.append(dict(uniq=uniq[o2], start=start[o2], cnt=cnt[o2],
                            src=bs, eid=be))
    return buckets


def _plan(per_core_all):
    """Shared (max-over-cores) call plan per bucket."""
    plans = []
    for b in range(NBUCK):
        core_widths = []
        for pc in per_core_all:
            bk = pc[b]
            ng = len(bk["uniq"])
            w = [int(bk["cnt"][g0:g0 + 128].max())
                 for g0 in range(0, ng, 128)] if ng else []
            core_widths.append(w)
        ngroups = max((len(w) for w in core_widths), default=0)
        gw = [max((w[g] if g < len(w) else 0) for w in core_widths) or 1
              for g in range(ngroups)]
        calls, cur = [], []
        for g in range(ngroups):
            if cur and sum(gw[x] for x in cur) + gw[g] > MAXCOL_CALL:
                calls.append(cur)
                cur = []
            cur.append(g)
        if cur:
            calls.append(cur)
        plan = [dict(glist=gl, widths=[gw[g] for g in gl],
                     cols=sum(gw[g] for g in gl)) for gl in calls]
        plans.append(dict(calls=plan, ngroups=ngroups))
    return plans


def _fill_core(plans, pc):
    """Per-core index arrays + edge-position maps following the shared plan."""
    out = []
    for b in range(NBUCK):
        bk = pc[b]
        ng = len(bk["uniq"])
        bcalls = []
        for call in plans[b]["calls"]:
            npos = call["cols"] * 128
            gidx = np.full(npos, ZREL, dtype=np.int16)
            sidx = np.empty(len(call["glist"]) * 128, dtype=np.int16)
            emap = np.full((128, call["cols"]), -1, dtype=np.int64)
            colbase = 0
            for gi_, g in enumerate(call["glist"]):
                w = call["widths"][gi_]
                for p in range(128):
                    u = g * 128 + p
                    if u < ng:
                        sidx[gi_ * 128 + p] = bk["uniq"][u]
                        s0 = bk["start"][u]
                        k = bk["cnt"][u]
                        gidx[(colbase + np.arange(k)) * 128 + p] = \
                            bk["src"][s0:s0 + k]
                        emap[p, colbase:colbase + k] = bk["eid"][s0:s0 + k]
                    else:
                        sidx[gi_ * 128 + p] = ZREL + (u % 44)
                colbase += w
            bcalls.append(dict(gidx=gidx, sidx=sidx, emap=emap))
        out.append(bcalls)
    return out


def _wrap16(idx):
    w = idx.reshape(-1, 16).T.copy()
    return np.tile(w, (8, 1))


# -------------------------------------------------------------- device build
def _build(plans_s, plans_t):
    import concourse.bacc as bacc
    import concourse.bass as bass
    import concourse.tile as tile
    import concourse.mybir as mybir

    f32 = mybir.dt.float32
    i16 = mybir.dt.int16
    AL = mybir.AluOpType
    ACT = mybir.ActivationFunctionType
    RG = [list(range(NCORES))]

    nc = bacc.Bacc("TRN2", debug=False, dynamic_dma_scratch_size=32768,
                   num_swdge_queues=4)

    gs = []
    for G, plans in (("s", plans_s), ("t", plans_t)):
        tgc = [sum(cl["cols"] for cl in plans[b]["calls"]) for b in range(NBUCK)]
        tgr = [sum(len(cl["glist"]) for cl in plans[b]["calls"])
               for b in range(NBUCK)]
        gs.append(dict(
            name=G, plans=plans, tgc=tgc,
            xT=nc.dram_tensor(f"xT_{G}", [128, SHP], f32, kind="ExternalInput"),
            W1=nc.dram_tensor(f"W1_{G}", [128, 128], f32, kind="ExternalInput"),
            b1=nc.dram_tensor(f"b1t_{G}", [128, 128], f32, kind="ExternalInput"),
            W3=nc.dram_tensor(f"W3_{G}", [128, 64], f32, kind="ExternalInput"),
            b3=nc.dram_tensor(f"b3t_{G}", [128, 64], f32, kind="ExternalInput"),
            deg=nc.dram_tensor(f"deg_{G}", [128, GROUPS], f32, kind="ExternalInput"),
            gidx=[nc.dram_tensor(f"gidx_{G}{b}", [128, max(tgc[b], 1) * 8], i16,
                                 kind="ExternalInput") for b in range(NBUCK)],
            sidx=[nc.dram_tensor(f"sidx_{G}{b}", [128, max(tgr[b], 1) * 8], i16,
                                 kind="ExternalInput") for b in range(NBUCK)],
            z_out=nc.dram_tensor(f"z_{G}", [SHP, 64], f32, kind="ExternalOutput"),
            ps_out=[nc.dram_tensor(f"ps_{G}{b}", [128, max(tgc[b], 1)], f32,
                                   kind="ExternalOutput") for b in range(NBUCK)],
        ))
    ident_d = nc.dram_tensor("ident", [128, 128], f32, kind="ExternalInput")

    qn = [0]

    def nextq():
        qn[0] = (qn[0] + 1) % 4
        return qn[0]

    with tile.TileContext(nc) as tc:
        with (
            tc.tile_pool(name="const", bufs=1) as constp,
            tc.tile_pool(name="dense", bufs=4) as densep,
            tc.tile_pool(name="psum", bufs=1, space="PSUM") as psump,
            tc.tile_pool(name="gat", bufs=3) as gatp,
            tc.tile_pool(name="red", bufs=3) as redp,
            tc.tile_pool(name="idx", bufs=6) as idxp,
            tc.tile_pool(name="psg", bufs=1) as psgp,
            tc.tile_pool(name="dram", bufs=1, space="DRAM") as dramp,
        ):
            ident = constp.tile([128, 128], f32, tag="ident")
            nc.sync.dma_start(ident[:], ident_d.ap())

            for g in gs:
                G = g["name"]
                plans = g["plans"]
                # constants
                W1s = constp.tile([128, 128], f32, tag=f"W1{G}")
                nc.sync.dma_start(W1s[:], g["W1"].ap())
                W3s = constp.tile([128, 64], f32, tag=f"W3{G}")
                nc.sync.dma_start(W3s[:], g["W3"].ap())
                b1s = constp.tile([128, 128], f32, tag=f"b1{G}")
                nc.sync.dma_start(b1s[:], g["b1"].ap())
                b3s = constp.tile([128, 64], f32, tag=f"b3{G}")
                nc.sync.dma_start(b3s[:], g["b3"].ap())
                degs = constp.tile([128, GROUPS], f32, tag=f"deg{G}")
                nc.sync.dma_start(degs[:], g["deg"].ap())
                dis = constp.tile([128, GROUPS], f32, tag=f"dis{G}")
                nc.vector.reciprocal(dis[:], degs[:])
                nc.scalar.activation(dis[:], dis[:], ACT.Sqrt)

                bounce1 = dramp.tile([SHP, 128], f32, tag=f"bn1{G}")
                bounce2 = dramp.tile([SHP, 64], f32, tag=f"bn2{G}")
                bounce3 = dramp.tile([SHP, 64], f32, tag=f"bn3{G}")
                hg1 = nc.dram_tensor(f"hg1_{G}", [NTOT, 128], f32,
                                     kind="Internal", addr_space="Shared").ap()
                hg2 = nc.dram_tensor(f"hg2_{G}", [NTOT, 64], f32,
                                     kind="Internal", addr_space="Shared").ap()
                zg = nc.dram_tensor(f"zg_{G}", [NTOT, 64], f32,
                                    kind="Internal", addr_space="Shared").ap()
                acc1 = dramp.tile([SHP, 128], f32, tag=f"ac1{G}")
                acc2 = dramp.tile([SHP, 64], f32, tag=f"ac2{G}")
                g.update(bounce2=bounce2, bounce3=bounce3, hg2=hg2, zg=zg,
                         acc1=acc1, acc2=acc2, dis=dis, b1s=b1s, b3s=b3s,
                         W3s=W3s, ident=ident)

                # ---- dense layer 1: h0' = dis * (x @ W1), write bounce + acc
                for j in range(GROUPS):
                    xt_t = densep.tile([128, 128], f32, tag="xt")
                    nc.sync.dma_start(xt_t[:], g["xT"].ap()[:, j * 128:(j + 1) * 128])
                    p1 = psump.tile([128, 128], f32, tag="mm1")
                    nc.tensor.matmul(p1[:], W1s[:], xt_t[:])
                    t0 = densep.tile([128, 128], f32, tag="t0")
                    nc.vector.tensor_copy(t0[:], p1[:])
                    p2 = psump.tile([128, 128], f32, tag="tr1")
                    nc.tensor.transpose(p2[:], t0[:], ident[:])
                    hp = densep.tile([128, 128], f32, tag="hp")
                    nc.scalar.mul(hp[:], p2[:], dis[:, j:j + 1])
                    nc.sync.dma_start(bounce1[j * 128:(j + 1) * 128, :], hp[:])
                    nc.sync.dma_start(acc1[j * 128:(j + 1) * 128, :], hp[:])
                nc.gpsimd.collective_compute(
                    "AllGather", AL.bypass, replica_groups=RG,
                    ins=[bounce1.opt()], outs=[hg1])
                g["hg1"] = hg1

            for g in gs:
                _edge_phase(nc, tc, g, g["hg1"], g["acc1"], 128, g["gidx"],
                            gatp, redp, idxp, nextq, layer=1)

            for g in gs:
                G = g["name"]
                plans = g["plans"]
                dis, b1s, b3s, W3s, ident = (g["dis"], g["b1s"], g["b3s"],
                                             g["W3s"], g["ident"])
                # ---- epilogue 1 + dense layer 2
                for j in range(GROUPS):
                    a1 = densep.tile([128, 128], f32, tag="a1")
                    nc.sync.dma_start(a1[:], g["acc1"][j * 128:(j + 1) * 128, :])
                    h1 = densep.tile([128, 128], f32, tag="h1")
                    nc.vector.scalar_tensor_tensor(
                        h1[:], a1[:], dis[:, j:j + 1], b1s[:],
                        op0=AL.mult, op1=AL.add)
                    h1r = densep.tile([128, 128], f32, tag="h1r")
                    nc.scalar.activation(h1r[:], h1[:], ACT.Relu)
                    p3 = psump.tile([128, 128], f32, tag="tr2")
                    nc.tensor.transpose(p3[:], h1r[:], ident[:])
                    h1T = densep.tile([128, 128], f32, tag="h1T")
                    nc.vector.tensor_copy(h1T[:], p3[:])
                    p4 = psump.tile([64, 128], f32, tag="mm2")
                    nc.tensor.matmul(p4[:], W3s[:], h1T[:])
                    t4 = densep.tile([64, 128], f32, tag="t4")
                    nc.vector.tensor_copy(t4[:], p4[:])
                    p5 = psump.tile([128, 64], f32, tag="tr3")
                    nc.tensor.transpose(p5[:], t4[:], ident[0:64, 0:64])
                    h2p = densep.tile([128, 64], f32, tag="h2p")
                    nc.scalar.mul(h2p[:], p5[:], dis[:, j:j + 1])
                    nc.sync.dma_start(g["bounce2"][j * 128:(j + 1) * 128, :], h2p[:])
                    nc.sync.dma_start(g["acc2"][j * 128:(j + 1) * 128, :], h2p[:])
                nc.gpsimd.collective_compute(
                    "AllGather", AL.bypass, replica_groups=RG,
                    ins=[g["bounce2"].opt()], outs=[g["hg2"]])

            for g in gs:
                _edge_phase(nc, tc, g, g["hg2"], g["acc2"], 64, g["gidx"],
                            gatp, redp, idxp, nextq, layer=2)

            for g in gs:
                dis, b3s = g["dis"], g["b3s"]
                for j in range(GROUPS):
                    a2 = densep.tile([128, 64], f32, tag="a2")
                    nc.sync.dma_start(a2[:], g["acc2"][j * 128:(j + 1) * 128, :])
                    zs = densep.tile([128, 64], f32, tag="zs")
                    nc.vector.scalar_tensor_tensor(
                        zs[:], a2[:], dis[:, j:j + 1], b3s[:],
                        op0=AL.mult, op1=AL.add)
                    nc.sync.dma_start(g["z_out"].ap()[j * 128:(j + 1) * 128, :], zs[:])
                    nc.sync.dma_start(g["bounce3"][j * 128:(j + 1) * 128, :], zs[:])
                nc.gpsimd.collective_compute(
                    "AllGather", AL.bypass, replica_groups=RG,
                    ins=[g["bounce3"].opt()], outs=[g["zg"]])

            for g in gs:
                _decoder(nc, tc, g, gatp, redp, idxp, psgp, nextq)

    nc.compile()
    return nc


def _edge_phase(nc, tc, g, table, acc, elem, gidx_d, gatp, redp, idxp, nextq,
                layer):
    import concourse.mybir as mybir
    AL = mybir.AluOpType
    f32 = mybir.dt.float32
    i16 = mybir.dt.int16
    for b in range(NBUCK):
        plans = g["plans"][b]
        goff = 0
        soff = 0
        tab_ap = table[b * BUCK:(b + 1) * BUCK, :]
        for call in plans["calls"]:
            cols = call["cols"]
            ngr = len(call["glist"])
            npos = cols * 128
            git = idxp.tile([128, cols * 8], i16, tag="gi")
            nc.sync.dma_start(git[:], g["gidx"][b].ap()[:, goff * 8:(goff + cols) * 8])
            gt = gatp.tile([128, MAXCOL_CALL, elem], f32, tag=f"gt{elem}")
            nc.gpsimd.dma_gather(gt[:, 0:cols, :], tab_ap, git[:], npos, npos,
                                 elem, single_packet=False, queue_num=nextq())
            rt = redp.tile([128, MAXCOL_CALL, elem], f32, tag=f"rt{elem}")
            c0 = 0
            for gi_, w in enumerate(call["widths"]):
                src = gt[:, c0:c0 + w, :].rearrange("p c f -> p f c")
                nc.vector.tensor_reduce(rt[:, gi_:gi_ + 1, :], src,
                                        axis=mybir.AxisListType.X, op=AL.add)
                c0 += w
            sit = idxp.tile([128, ngr * 8], i16, tag="si")
            nc.sync.dma_start(sit[:], g["sidx"][b].ap()[:, soff * 8:(soff + ngr) * 8])
            nc.gpsimd.dma_scatter_add(acc[:], rt[:, 0:ngr, :], sit[:],
                                      ngr * 128, ngr * 128, elem,
                                      single_packet=False, queue_num=nextq())
            goff += cols
            soff += ngr


def _decoder(nc, tc, g, gatp, redp, idxp, psgp, nextq):
    import concourse.bass as bass
    import concourse.mybir as mybir
    AL = mybir.AluOpType
    ACT = mybir.ActivationFunctionType
    f32 = mybir.dt.float32
    i16 = mybir.dt.int16
    G = g["name"]
    for b in range(NBUCK):
        plans = g["plans"][b]
        tgc = g["tgc"][b]
        psg = psgp.tile([128, max(tgc, 1)], f32, tag=f"psg{G}{b}")
        goff = 0
        soff = 0
        tab_ap = g["zg"][b * BUCK:(b + 1) * BUCK, :]
        for call in plans["calls"]:
            cols = call["cols"]
            ngr = len(call["glist"])
            npos = cols * 128
            git = idxp.tile([128, cols * 8], i16, tag="gi")
            nc.sync.dma_start(git[:], g["gidx"][b].ap()[:, goff * 8:(goff + cols) * 8])
            zsrc = gatp.tile([128, MAXCOL_CALL, 64], f32, tag="gt64")
            nc.gpsimd.dma_gather(zsrc[:, 0:cols, :], tab_ap, git[:], npos, npos,
                                 64, single_packet=False, queue_num=nextq())
            sit = idxp.tile([128, ngr * 8], i16, tag="si")
            nc.sync.dma_start(sit[:], g["sidx"][b].ap()[:, soff * 8:(soff + ngr) * 8])
            zdst = redp.tile([128, MAXCOL_CALL, 64], f32, tag="rt64")
            nc.gpsimd.dma_gather(zdst[:, 0:ngr, :], g["bounce3"][:], sit[:],
                                 ngr * 128, ngr * 128, 64,
                                 single_packet=False, queue_num=nextq())
            prod = gatp.tile([128, MAXCOL_CALL, 64], f32, tag="prod")
            c0 = 0
            for gi_, w in enumerate(call["widths"]):
                a_ap, b_ap = bass.broadcast_tensor_aps(
                    zsrc[:, c0:c0 + w, :], zdst[:, gi_:gi_ + 1, :])
                nc.vector.tensor_mul(prod[:, c0:c0 + w, :], a_ap, b_ap)
                c0 += w
            nc.vector.tensor_reduce(psg[:, goff:goff + cols],
                                    prod[:, 0:cols, :],
                                    axis=mybir.AxisListType.X, op=AL.add)
            nc.scalar.activation(psg[:, goff:goff + cols],
                                 psg[:, goff:goff + cols], ACT.Sigmoid)
            goff += cols
            soff += ngr
        nc.sync.dma_start(g["ps_out"][b].ap(), psg[:])


# ------------------------------------------------------------------- kernel
def kernel(xs, xt, s_edge_index, t_edge_index, W1, b1, W2, b2, W3, b3):
    from concourse.bass_utils import run_bass_kernel_spmd

    xs = np.asarray(xs, dtype=np.float32)
    xt = np.asarray(xt, dtype=np.float32)
    deg_s, pcs = _prep_graph(s_edge_index)
    deg_t, pct = _prep_graph(t_edge_index)
    plans_s = _plan(pcs)
    plans_t = _plan(pct)

    key = (tuple((c["cols"], len(c["glist"])) for b in range(NBUCK)
                 for c in plans_s[b]["calls"]),
           tuple((c["cols"], len(c["glist"])) for b in range(NBUCK)
                 for c in plans_t[b]["calls"]))
    if key not in _cache:
        _cache.clear()
        _cache[key] = _build(plans_s, plans_t)
    nc = _cache[key]

    fill_s = [_fill_core(plans_s, pcs[c]) for c in range(NCORES)]
    fill_t = [_fill_core(plans_t, pct[c]) for c in range(NCORES)]

    def degw(deg, c):
        d = np.full(SHP, np.inf, dtype=np.float32)
        d[:NSH] = deg[c * NSH:(c + 1) * NSH]
        return d.reshape(GROUPS, 128).T.copy()

    def xTsh(x, c):
        v = np.zeros((128, SHP), dtype=np.float32)
        v[:, :NSH] = x[c * NSH:(c + 1) * NSH].T
        return v

    in_maps = []
    for c in range(NCORES):
        m = {"ident": np.eye(128, dtype=np.float32)}
        for G, x, W, bb, deg, fill, plans in (
                ("s", xs, W1, b1, deg_s, fill_s, plans_s),
                ("t", xt, W2, b2, deg_t, fill_t, plans_t)):
            m[f"xT_{G}"] = xTsh(x, c)
            m[f"W1_{G}"] = np.asarray(W, dtype=np.float32)
            m[f"b1t_{G}"] = np.tile(np.asarray(bb, np.float32)[None, :], (128, 1))
            m[f"W3_{G}"] = np.asarray(W3, dtype=np.float32)
            m[f"b3t_{G}"] = np.tile(np.asarray(b3, np.float32)[None, :], (128, 1))
            m[f"deg_{G}"] = degw(deg, c)
            for b in range(NBUCK):
                bcalls = fill[c][b]
                if bcalls:
                    gi = np.concatenate([_wrap16(cl["gidx"]) for cl in bcalls], axis=1)
                    si = np.concatenate([_wrap16(cl["sidx"]) for cl in bcalls], axis=1)
                else:
                    gi = np.zeros((128, 8), np.int16)
                    si = np.zeros((128, 8), np.int16)
                m[f"gidx_{G}{b}"] = gi
                m[f"sidx_{G}{b}"] = si
        in_maps.append(m)

    res = run_bass_kernel_spmd(nc, in_maps, core_ids=list(range(NCORES)),
                               trace=TRACE)
    LAST_EXEC_NS[0] = res.exec_time_ns
    outs = res.results

    zs = np.concatenate([outs[c]["z_s"][:NSH] for c in range(NCORES)], axis=0)
    zt = np.concatenate([outs[c]["z_t"][:NSH] for c in range(NCORES)], axis=0)

    def assemble_ps(G, fill, plans):
        ps = np.zeros(E, dtype=np.float32)
        for c in range(NCORES):
            for b in range(NBUCK):
                grid = outs[c][f"ps_{G}{b}"]
                goff = 0
                for cl in fill[c][b]:
                    emap = cl["emap"]
                    cols = emap.shape[1]
                    valid = emap >= 0
                    ps[emap[valid]] = grid[:, goff:goff + cols][valid]
                    goff += cols
        return ps

    ps = assemble_ps("s", fill_s, plans_s)
    pt = assemble_ps("t", fill_t, plans_t)
    return zs, zt, ps, pt
